# revision 79
# baseline (speedup 1.0000x reference)
"""Trainium2 Bass kernel for KNN-masked multi-head agent-agent attention.

Problem (per scene): N=1024 agents, D=256 model dim, H=4 heads, K=32 nearest
neighbours by distance. Full pipeline:
    top-K mask from distances -> additive bias (-d/50, -inf outside mask)
    -> MHA (shared in-proj, softmax, out-proj) -> residual + LayerNorm.

Sharding: data-parallel over the batch axis B=8 -> one scene per NeuronCore
(8 cores), no collectives. Each core runs the identical program (SPMD) on its
own scene; the host stacks per-core outputs.

Per-core design. Every engine executes its stream in order, so streams are
kept phase-pure — a stalled op poisons everything emitted after it on the
same engine — and all input tensors are fetched with a handful of batched
DMAs so the HWDGE generator doesn't serialize the front of the schedule.

  * Selection (DVE): 4 rounds of (max8 + match_replace imm=-1e30) on nd=-d
    mark the exact top-32 multiset in-place — match_replace replaces the
    lowest-index occurrence of each of the 8 values per round, which
    reproduces jax.lax.top_k's index tie-breaking exactly for any tie
    multiplicity (the dataset contains exact f32 ties at the 32/33 boundary,
    so this matters). nd conversions are hoisted to the front of the Act
    stream; distance loads go first on the SP DMA queue.
  * Bias (Act+Pool, negative-offset form, fp16):
    bias = 0.02*nd - 44*[not selected]. Selected entries carry only 0.02*nd
    (full fp16 precision); masked entries sit near -44 where precision is
    irrelevant and exp underflows fp16 to exactly 0 — so no exp offset and
    no -inf handling are needed. The mask is decoded off the DVE stream with
    Pool-legal opcodes only: t = sc * 4.4e-29 maps the -1e30 selection
    marker to exactly -44 and every surviving score (> -100) to ~0, so
    bias = (0.02*nd - 44) - t via one Act scale-and-bias plus two Pool
    tensor_tensor ops. The natural-layout fp16 bias is transposed by the
    DMA xbar (dma_start_transpose) into per-key-block layout, one tensor
    per query tile (a shared tensor would serialize the transposes behind
    prior chunks' matmul reads at tile granularity).
  * Attention (PE/Act), interleaved with selection in query chunks lagging
    two tiles behind: S^T = K_h Q_h^T (f32r, 1 cycle/row) accumulated with an
    identity-matmul of the transposed fp16 bias, probs = exp(PSUM) in fp16
    with one activation per 4-key-block PSUM group. The AV matmul uses the
    fp16 probability blocks as STATIONARY and the ones-augmented V as moving,
    which (a) halves the moving-row count and (b) lands the result in
    natural layout av2[q, h, 0:64] with the softmax denominator at
    av2[q, h, 64] — a per-partition scalar.
  * Normalize + out-proj, deferred one chunk so no stream waits: av2 stages
    through SBUF (Act; GPSIMD cannot read PSUM and has no tensor_scalar
    opcode), DVE takes a [128,4] reciprocal and 4 ptr-scalar multiplies,
    the fp16 rows go back through the DMA xbar to head-major layout, and the
    out-projection (fp16 weights) + residual completes per tile. The V
    in-proj bias is never added to V: softmax weights sum to 1, so its
    contribution bv @ Wo^T is folded into the output-bias constant.
  * LayerNorm epilogue: stage-batched at the end (bn_stats/aggr + scale on
    DVE, sqrt on Act, beta-add on Pool), stores on the Act DMA queue.
"""

import os
import sys
import numpy as np

sys.path.insert(0, "/opt/trn_rl_repo")

import concourse.bass as bass
import concourse.tile as tile
from concourse import mybir
from concourse.masks import make_identity

f32 = mybir.dt.float32
f32r = mybir.dt.float32r
f16 = mybir.dt.float16
Alu = mybir.AluOpType
Act = mybir.ActivationFunctionType

N = 1024
D = 256
H = 4
HD = 64
NT = N // 128          # 8 query/token tiles
KB = N // 128          # 8 key blocks
D_REF = 50.0
LN_EPS = 1e-5
NEG_BIG = -1.0e30
MASK_M = -44.0         # additive mask for non-selected entries (exp -> 0)

MM_DT = f32r


def build_nc(K: int, split_waits: bool = True):
    nc = bass.Bass("TRN2", target_bir_lowering=False, debug=False)

    x_d = nc.dram_tensor("repr1", [N, D], f32, kind="ExternalInput").ap()
    d_d = nc.dram_tensor("distances", [N, N], f32, kind="ExternalInput").ap()
    wi_d = nc.dram_tensor("in_proj_w", [3 * D, D], f32, kind="ExternalInput").ap()
    bi_d = nc.dram_tensor("in_proj_b", [3 * D], f32, kind="ExternalInput").ap()
    wo_d = nc.dram_tensor("out_proj_w", [D, D], f32, kind="ExternalInput").ap()
    bo_d = nc.dram_tensor("out_proj_b", [D], f32, kind="ExternalInput").ap()
    g_d = nc.dram_tensor("ln_gamma", [D], f32, kind="ExternalInput").ap()
    be_d = nc.dram_tensor("ln_beta", [D], f32, kind="ExternalInput").ap()
    out_d = nc.dram_tensor("out", [N, D], f32, kind="ExternalOutput").ap()

    with tile.TileContext(nc) as tc:
        _emit(tc, K, x_d, d_d, wi_d, bi_d, wo_d, bo_d, g_d, be_d, out_d)
    if split_waits:
        _split_waits(nc)
    return nc


def _split_waits(nc, max_waits: int = 1):
    """Walrus codegen rejects instructions carrying more than one sync wait
    (e.g. transpose-matmul LDW structs and HWDGE DMA descriptors), and the
    DMA_DIRECT2D_XPOSE struct carries none at all. Move the excess waits onto
    engine NoOps issued immediately before — the sequencer stalls on those
    first, which is semantically identical."""
    k = 0
    for fn in nc.m.functions:
        for blk in fn.blocks:
            new = []
            for ins in blk.instructions:
                si = ins.sync_info
                mw = 0 if isinstance(ins, mybir.InstDmaTransposeAnt) else max_waits
                if si is not None and si.on_wait and len(si.on_wait) > mw:
                    waits = list(si.on_wait)
                    keep = waits[-mw:] if mw else []
                    for w in (waits[:-mw] if mw else waits):
                        nop = mybir.InstNoOp(
                            name=f"I-wsplit-{k}", engine=ins.engine)
                        nop.sync_info = mybir.SyncInfo(on_wait=[w], on_update=[])
                        new.append(nop)
                        k += 1
                    ins.sync_info = mybir.SyncInfo(
                        on_wait=keep, on_update=list(si.on_update))
                new.append(ins)
            blk.instructions[:] = new


def _bcast_dram_row(nc, dst, src_ap, offset, width):
    """DMA-replicate a [width] DRAM row into all 128 partitions of dst."""
    rep = bass.AP(
        tensor=src_ap.tensor,
        offset=src_ap.offset + offset,
        ap=[[0, 128], [1, width]],
    )
    nc.gpsimd.dma_start(out=dst, in_=rep)


def _emit(tc, K, x_d, d_d, wi_d, bi_d, wo_d, bo_d, g_d, be_d, out_d):
    from contextlib import ExitStack
    nc = tc.nc
    ctx = ExitStack()

    consts = ctx.enter_context(tc.tile_pool(name="consts", bufs=1))
    persist = ctx.enter_context(tc.tile_pool(name="persist", bufs=1))
    dstage = ctx.enter_context(tc.tile_pool(name="dstage", bufs=2))
    dpre = ctx.enter_context(tc.tile_pool(name="dpre", bufs=2))
    ndp = ctx.enter_context(tc.tile_pool(name="ndp", bufs=5))
    selp = ctx.enter_context(tc.tile_pool(name="selp", bufs=4))
    ptp = ctx.enter_context(tc.tile_pool(name="ptp", bufs=3))
    epi = ctx.enter_context(tc.tile_pool(name="epi", bufs=5))
    ps_s = ctx.enter_context(tc.tile_pool(name="ps_s", bufs=2, space="PSUM"))
    ps_av = ctx.enter_context(tc.tile_pool(name="ps_av", bufs=2, space="PSUM"))
    ps_tr = ctx.enter_context(tc.tile_pool(name="ps_tr", bufs=1, space="PSUM"))
    ps_o = ctx.enter_context(tc.tile_pool(name="ps_o", bufs=1, space="PSUM"))

    # sc * CTINY maps the -1e30 selection marker to exactly -MASK_M and
    # every real (negative, > -100) score to ~0: a Pool-legal multiplicative
    # decode of the match_replace marking.
    ctiny = consts.tile([128, N], f32, name="ctiny")
    nc.gpsimd.memset(ctiny, -MASK_M * 1.0e-30)
    cm44 = consts.tile([128, 1], f32, name="cm44")
    nc.gpsimd.memset(cm44, MASK_M)

    # ------- distance prefetch (SP queue first) + upfront nd conversion ----
    # batched 2-tile DMAs: each HWDGE generation costs ~650ns serial, so the
    # fewer DMA instructions issued ahead of the bias transposes, the better
    nds = [ndp.tile([128, N], f32, name="nd", tag="nd") for i in range(NT)]
    # tile 0 rides alone, and its negation runs on the DVE into a tile the
    # DVE alone owns, so the first max8 needs no Act round-trip and no other
    # engine ever writes what the selection reads. nds[0] instead receives
    # tile 0's bias precursor bm = -0.02*d - 44 directly from the raw
    # distances on Act (single writer per tile — race-hardened).
    d0 = dpre.tile([128, N], f32, name="d0", tag="d0")
    nc.sync.dma_start(out=d0, in_=d_d[0:128, :])
    nd0d = consts.tile([128, N], f32, name="nd0d")
    nc.vector.tensor_scalar_mul(nd0d, d0, -1.0)  # nd = -d (DVE-owned)
    # tile 0's bias precursor bm = -0.02*d - 44, emitted FIRST on Act so the
    # later nd conversions (which rotate through nds[0]'s slot) can't
    # head-block the Act stream against biasf-0
    nc.scalar.activation(nds[0], d0, Act.Identity, scale=-1.0 / D_REF,
                         bias=cm44)
    d1 = dpre.tile([128, N], f32, name="d1", tag="d0")
    nc.sync.dma_start(out=d1, in_=d_d[128:256, :])
    nc.scalar.activation(nds[1], d1, Act.Copy, scale=-1.0)
    for g in range(3):
        dpair = dpre.tile([128, 2, N], f32, name="dpair", tag="dpair")
        nc.sync.dma_start(out=dpair, in_=d_d[(2 + 2 * g) * 128:(4 + 2 * g) * 128, :]
                          .rearrange("(t p) k -> p t k", t=2))
        for u in range(2):
            nc.scalar.activation(nds[2 + 2 * g + u], dpair[:, u, :], Act.Copy,
                                 scale=-1.0)

    # in-proj bias first (tiny; its Q slices are rescaled before selection)
    bibig = consts.tile([128, 6], f32, name="bibig")
    nc.sync.dma_start(out=bibig, in_=bi_d.rearrange("(r p) -> p r", p=128))

    # -------- remaining input loads: one batched DMA per tensor ------------
    wbig = consts.tile([128, 6, D], f32, name="wbig")
    nc.sync.dma_start(out=wbig, in_=wi_d.rearrange("(r p) e -> p r e", p=128))
    wobig = consts.tile([128, 2, D], f32, name="wobig")
    nc.sync.dma_start(out=wobig, in_=wo_d.rearrange("(r p) e -> p r e", p=128))
    xbig = persist.tile([128, NT, D], f32, name="xbig")
    nc.sync.dma_start(out=xbig, in_=x_d.rearrange("(r p) e -> p r e", p=128))

    wrows = [wbig[:, r, :] for r in range(6)]
    worows = [wobig[:, r, :] for r in range(2)]
    xrows = [xbig[:, i, :] for i in range(NT)]
    bqk = [bibig[:, mb:mb + 1] for mb in range(4)]
    bvcol32 = [bibig[:, 4 + c:5 + c] for c in range(2)]

    # ---------------- constants (Pool) ----------------
    ident = consts.tile([128, 128], f32, name="ident")
    make_identity(nc, ident)
    identh = consts.tile([128, 128], f16, name="identh")
    nc.gpsimd.tensor_copy(identh, ident)
    identwarm = ps_tr.tile([128, 128], f32, name="identwarm", tag="wtr")
    nc.tensor.matmul(identwarm, lhsT=ident, rhs=ident, is_transpose=True)

    epsc = consts.tile([128, 1], f32, name="epsc")
    nc.gpsimd.memset(epsc, LN_EPS)
    onesrow = consts.tile([1, 128], f16, name="onesrow")
    nc.gpsimd.memset(onesrow, 1.0)
    bvcol = []
    for c in range(2):
        t = consts.tile([128, 1], f16, name=f"bvcol{c}")
        nc.gpsimd.tensor_copy(t, bvcol32[c])
        bvcol.append(t)

    bo_b = consts.tile([128, D], f32, name="bo_b")
    _bcast_dram_row(nc, bo_b, bo_d, 0, D)
    g_b = consts.tile([128, D], f32, name="g_b")
    _bcast_dram_row(nc, g_b, g_d, 0, D)
    be_b = consts.tile([128, D], f32, name="be_b")
    _bcast_dram_row(nc, be_b, be_d, 0, D)

    # -------- weights / X: PE transposes + Act copies (after the nds) ------
    wt = [persist.tile([128, 3 * D], f32, name=f"wt{c}") for c in range(2)]
    for r in range(6):
        for c in range(2):
            pt = ps_tr.tile([128, 128], f32, name="wtr", tag="wtr")
            nc.tensor.matmul(pt, lhsT=wrows[r][:, c * 128:(c + 1) * 128],
                             rhs=ident, is_transpose=True)
            nc.scalar.activation(wt[c][:, r * 128:(r + 1) * 128].bitcast(f32r),
                                 pt, Act.Copy)

    wot = [persist.tile([128, D], f16, name=f"wot{c}") for c in range(2)]
    for r in range(2):
        for c in range(2):
            pt = ps_tr.tile([128, 128], f32, name="wotr", tag="wtr")
            nc.tensor.matmul(pt, lhsT=worows[r][:, c * 128:(c + 1) * 128],
                             rhs=ident, is_transpose=True)
            nc.scalar.activation(wot[c][:, r * 128:(r + 1) * 128], pt, Act.Copy)


    xt = [persist.tile([128, N], f32, name=f"xt{c}") for c in range(2)]
    for i in range(NT):
        for c in range(2):
            pt = ps_tr.tile([128, 128], f32, name="xtr", tag="wtr")
            nc.tensor.matmul(pt, lhsT=xrows[i][:, c * 128:(c + 1) * 128],
                             rhs=ident, is_transpose=True)
            nc.scalar.activation(xt[c][:, i * 128:(i + 1) * 128].bitcast(f32r),
                                 pt, Act.Copy)

    # ---------------- Q^T, K^T, V ----------------
    qkt = [persist.tile([128, N], f32, name=f"qkt{mb}") for mb in range(4)]
    for mb in range(4):
        for qc in range(4):
            ps = ps_o.tile([128, D], f32, name="qk_ps", tag="ps_o")
            for c in range(2):
                nc.tensor.matmul(
                    ps,
                    lhsT=wt[c][:, mb * 128:(mb + 1) * 128].bitcast(MM_DT),
                    rhs=xt[c][:, qc * 256:(qc + 1) * 256].bitcast(MM_DT),
                    start=(c == 0), stop=(c == 1))
            nc.scalar.activation(qkt[mb][:, qc * 256:(qc + 1) * 256].bitcast(f32r),
                                 ps, Act.Identity, bias=bqk[mb],
                                 scale=0.125 if mb < 2 else 1.0)

    # V padded per head, fp16: [128, H, 65]; col 64 of each head slot is the
    # ones column that produces the softmax denominator in the AV matmul.
    # V carries NO in-proj bias — softmax rows sum to 1, so the bias
    # contributes bv @ Wo^T to the output, folded into bo_full below.
    vpad = [persist.tile([128, H, HD + 1], f16, name=f"vpad{kb}") for kb in range(KB)]
    for kb in range(KB):
        nc.gpsimd.memset(vpad[kb][:, :, HD:HD + 1], 1.0)
        ps = ps_o.tile([128, D], f32, name="v_ps", tag="ps_o")
        for c in range(2):
            nc.tensor.matmul(
                ps,
                lhsT=xt[c][:, kb * 128:(kb + 1) * 128].bitcast(MM_DT),
                rhs=wt[c][:, 2 * D:3 * D].bitcast(MM_DT),
                start=(c == 0), stop=(c == 1))
        nc.scalar.activation(
            vpad[kb][:, :, 0:HD],
            ps.rearrange("p (h e) -> p h e", h=H), Act.Copy)


    # bo_full = bo + bv @ Wo^T, broadcast into all partitions.
    # (GPSIMD cannot read PSUM, so the final add runs on the DVE; it is
    # emitted between selection tiles where its inputs are long ready.)
    bvwo_ps = ps_o.tile([1, D], f32, name="bvwo_ps", tag="ps_o")
    for c in range(2):
        nc.tensor.matmul(bvwo_ps, lhsT=bvcol[c], rhs=wot[c],
                         start=(c == 0), stop=(c == 1))
    bvwo = consts.tile([1, D], f16, name="bvwo")
    nc.scalar.activation(bvwo, bvwo_ps, Act.Copy)
    bvwo_b = consts.tile([128, D], f32, name="bvwo_b")
    bvwo_bp = ps_o.tile([128, D], f32, name="bvwo_bp", tag="ps_o")
    nc.tensor.matmul(bvwo_bp, lhsT=onesrow, rhs=bvwo)
    nc.scalar.activation(bvwo_b, bvwo_bp, Act.Copy)
    bo_full = consts.tile([128, D], f32, name="bo_full")

    # ---------------- selection / bias / attention ------------------------
    # one transposed-bias tensor PER query tile: a shared tensor would
    # serialize each DMA transpose behind every prior chunk's matmul reads
    # (tile-granular write-after-read).
    bias_t = [persist.tile([128, KB, 128], f16, name=f"bias_t{i}")
              for i in range(NT)]
    xb = []
    for i in range(NT):
        xb.append(persist.tile([128, D], f32, name=f"xb{i}"))
    xs = []      # residual+attn rows awaiting LayerNorm


    def select_tile(i):
        # DVE: the exact top-32 multiset marking
        nd = nd0d if i == 0 else nds[i]
        m32 = selp.tile([128, 32], f32, name="m32", tag="m32")
        sc = selp.tile([128, N], f32, name="selsc", tag="selsc")
        nc.vector.max(m32[:, 0:8], nd)
        nc.vector.match_replace(sc, m32[:, 0:8], nd, NEG_BIG)
        nc.vector.max(m32[:, 8:16], sc)
        nc.vector.match_replace(sc, m32[:, 8:16], sc, NEG_BIG)
        nc.vector.max(m32[:, 16:24], sc)
        nc.vector.match_replace(sc, m32[:, 16:24], sc, NEG_BIG)
        nc.vector.max(m32[:, 24:32], sc)
        nc.vector.match_replace(sc, m32[:, 24:32], sc, NEG_BIG)
        # sc == NEG_BIG exactly marks the reference top-32 multiset.

        # Act: bm = 0.02*nd - 44;  Pool: t = sc*CTINY = {-44 sel, ~0 not};
        # Pool: biasf = bm - t = 0.02*nd - 44*[not selected]  (fp16 out).
        # This keeps the bias build entirely off the DVE selection stream.
        # nd is dead after the first match_replace read: reuse it for bm
        if i == 0:
            bm = nds[0]
        else:
            # nd is dead after the first match_replace read: reuse it for bm
            bm = nd
            nc.scalar.activation(bm, nd, Act.Identity, scale=1.0 / D_REF,
                                 bias=cm44)
        t = selp.tile([128, N], f32, name="tdec", tag="m40")
        nc.gpsimd.tensor_tensor(t, sc, ctiny, Alu.mult)
        biasf = selp.tile([128, N], f16, name="biasf", tag="biasf")
        nc.gpsimd.tensor_tensor(biasf, bm, t, Alu.subtract)
        # transpose into per-tile bias_t (runs on the DMA xbar)
        nc.sync.dma_start_transpose(out=bias_t[i], in_=biasf)

    def attn_chunk(q0, QW):
        # Attention with a NATURAL-layout output: the AV matmul uses the
        # probability blocks as stationary, yielding av2[q, h, 0:64] plus the
        # softmax denominator at av2[q, h, 64] — a per-partition scalar, so
        # normalization is a single Pool divide straight out of PSUM. The
        # normalized fp16 rows go back through the DMA xbar to the
        # head-major-transposed layout the out-projection needs.
        qs = slice(q0, q0 + QW)
        nu = QW // 128
        av2s = [ps_av.tile([128, H, HD + 1], f32, name="av2", tag="ps_av")
                for _ in range(nu)]
        for h in range(H):
            qmb, kmb = h // 2, 2 + h // 2
            p0 = (h % 2) * HD
            pt_groups = []
            for g in range(2):      # 2 groups of 4 key blocks
                psf = ps_s.tile([128, 4, 256], f32, name="s_ps", tag="ps_s")
                ps = psf[:, :, 0:QW]
                for j in range(4):
                    kb = 4 * g + j
                    nc.tensor.matmul(
                        ps[:, j, :],
                        lhsT=qkt[kmb][p0:p0 + HD, kb * 128:(kb + 1) * 128].bitcast(MM_DT),
                        rhs=qkt[qmb][p0:p0 + HD, qs].bitcast(MM_DT),
                        start=True, stop=False)
                    for u in range(nu):
                        it = q0 // 128 + u
                        nc.tensor.matmul(
                            ps[:, j, u * 128:(u + 1) * 128],
                            lhsT=identh, rhs=bias_t[it][:, kb, :],
                            start=False, stop=(u == nu - 1))
                ptgf = ptp.tile([128, 4, 256], f16, name="pt", tag="pt")
                ptg = ptgf[:, :, 0:QW]
                nc.scalar.activation(ptg, ps, Act.Exp)
                pt_groups.append(ptg)
            for u in range(nu):
                for kb in range(KB):
                    nc.tensor.matmul(
                        av2s[u][:, h, :],
                        lhsT=pt_groups[kb // 4][:, kb % 4, u * 128:(u + 1) * 128],
                        rhs=vpad[kb][:, h, :],
                        start=(kb == 0), stop=(kb == KB - 1))

        for u in range(nu):
            qt = q0 // 128 + u
            av2 = av2s[u]
            # GPSIMD cannot read PSUM: stage av2 through SBUF on Act; the
            # DVE-side normalize is deferred one chunk (finish_tiles) so the
            # in-order DVE stream never waits on this chunk's pipeline.
            avsb = epi.tile([128, H, HD + 1], f32, name="avsb", tag="avsb")
            nc.scalar.activation(avsb, av2, Act.Copy)
            pending.append((qt, avsb))

    def finish_tiles(n=NT):
        for _ in range(min(n, len(pending))):
            qt, avsb = pending.pop(0)
            rd4 = epi.tile([128, H], f32, name="rd4", tag="rd4")
            nc.vector.reciprocal(rd4, avsb[:, :, HD:HD + 1]
                                 .rearrange("p h o -> p (h o)"))
            attn_nat = epi.tile([128, D], f16, name="attn_nat", tag="anat")
            for h in range(H):
                nc.vector.tensor_scalar(
                    attn_nat[:, h * HD:(h + 1) * HD], avsb[:, h, 0:HD],
                    rd4[:, h:h + 1], None, Alu.mult)
            # back to head-major transposed layout for the out-projection
            # (issued on the Act DMA queue so bias transposes aren't blocked)
            attnt_qt = persist.tile([128, 2, 128], f16, name=f"attnt_qt{qt}")
            nc.scalar.dma_start_transpose(out=attnt_qt, in_=attn_nat)
            nc.gpsimd.tensor_tensor(xb[qt], xrows[qt], bo_full, Alu.add)
            po = ps_o.tile([128, D], f32, name="o_ps", tag="ps_o")
            for c in range(2):
                nc.tensor.matmul(
                    po,
                    lhsT=attnt_qt[:, c, :],
                    rhs=wot[c],
                    start=(c == 0), stop=(c == 1))
            posb = epi.tile([128, D], f32, name="posb", tag="posb")
            nc.scalar.activation(posb, po, Act.Copy)
            x = persist.tile([128, D], f32, name=f"x_epi{qt}")
            nc.gpsimd.tensor_tensor(x, posb, xb[qt], Alu.add)
            xs.append(x)

    # ---------------- LayerNorm epilogue (stage-batched, split so tiles
    # 0..5 normalize while the last chunk is still in flight) --------------
    lnp = ctx.enter_context(tc.tile_pool(name="lnp", bufs=1))
    sts = [lnp.tile([128, 6], f32, name=f"st{i}") for i in range(NT)]
    mvs = [lnp.tile([128, 2], f32, name=f"mv{i}") for i in range(NT)]
    sds = [lnp.tile([128, 1], f32, name=f"sd{i}") for i in range(NT)]
    rstds = [lnp.tile([128, 1], f32, name=f"rstd{i}") for i in range(NT)]

    def ln_emit(tbs):
        for tb in tbs:
            nc.vector.bn_stats(sts[tb], xs[tb])
            nc.vector.bn_aggr(mvs[tb], sts[tb])
        for tb in tbs:
            nc.scalar.activation(sds[tb], mvs[tb][:, 1:2], Act.Sqrt, bias=epsc)
        for tb in tbs:
            nc.vector.reciprocal(rstds[tb], sds[tb])
        for tb in tbs:
            nc.vector.tensor_scalar(xs[tb], xs[tb], mvs[tb][:, 0:1], None,
                                    Alu.subtract)
        for tb in tbs:
            nc.vector.scalar_tensor_tensor(
                out=xs[tb], in0=g_b, scalar=rstds[tb], in1=xs[tb],
                op0=Alu.mult, op1=Alu.mult)
        for tb in tbs:
            nc.gpsimd.tensor_tensor(xs[tb], xs[tb], be_b, Alu.add)
        for tb in tbs:
            nc.scalar.dma_start(out=out_d[tb * 128:(tb + 1) * 128, :],
                                in_=xs[tb])


    pending = []
    for mb in range(2):   # bias enters after the 1/8 activation scale
        nc.vector.tensor_scalar_mul(bqk[mb], bqk[mb], 0.125)
    select_tile(0)
    select_tile(1)
    nc.gpsimd.tensor_tensor(bo_full, bo_b, bvwo_b, Alu.add)
    select_tile(2)
    select_tile(3)
    attn_chunk(0, 256)
    select_tile(4)
    select_tile(5)
    attn_chunk(256, 256)
    finish_tiles(2)       # tiles 0,1 — fully drained chunks only, so the
    select_tile(6)        # in-order DVE stream never reaches a finish whose
    select_tile(7)        # attention is still in flight
    attn_chunk(512, 256)
    finish_tiles(2)       # tiles 2,3
    attn_chunk(768, 256)
    finish_tiles(2)       # tiles 4,5
    finish_tiles()        # tiles 6,7
    ln_emit(range(6))
    ln_emit(range(6, NT))

    ctx.close()


_NC_CACHE = {}


def _get_nc(K: int):
    if K not in _NC_CACHE:
        _NC_CACHE[K] = build_nc(K)
    return _NC_CACHE[K]


def kernel(**inputs) -> np.ndarray:
    from concourse.bass_utils import run_bass_kernel_spmd

    K = int(np.asarray(inputs["K"]))
    assert K == 32, f"kernel specialized for K=32, got {K}"
    B = inputs["repr1"].shape[0]
    nc = _get_nc(K)

    shared = {
        "in_proj_w": np.ascontiguousarray(inputs["in_proj_w"], np.float32),
        "in_proj_b": np.ascontiguousarray(inputs["in_proj_b"], np.float32),
        "out_proj_w": np.ascontiguousarray(inputs["out_proj_w"], np.float32),
        "out_proj_b": np.ascontiguousarray(inputs["out_proj_b"], np.float32),
        "ln_gamma": np.ascontiguousarray(inputs["ln_gamma"], np.float32),
        "ln_beta": np.ascontiguousarray(inputs["ln_beta"], np.float32),
    }
    in_maps = []
    for b in range(B):
        m = dict(shared)
        m["repr1"] = np.ascontiguousarray(inputs["repr1"][b], np.float32)
        m["distances"] = np.ascontiguousarray(inputs["distances"][b], np.float32)
        in_maps.append(m)

    res = run_bass_kernel_spmd(nc, in_maps, list(range(B)))
    out = np.stack([np.asarray(res.results[b]["out"]) for b in range(B)])
    return out.astype(np.float32)
